# revision 8
# baseline (speedup 1.0000x reference)
"""CVRP decoder (3-layer transformer + scatter) on 8 trn2 NeuronCores.

Self-contained: hardcodes shapes/sharding for
  nn_CVRP_Decoder (B=512, SEQ=102, EMBED=256, HEADS=16, DK=16, FF=1024, L=3).

Strategy: pure data parallel over batch (64 rows/core). Per core the
whole network runs feature-major ([256->2x128 partitions, tokens]) in
groups of 8 batch rows (816 tokens); attention is row-local. Matmuls in
bf16 (fp32 PSUM accumulate), residual stream in fp32. Scores S^T =
k_h^T q_h per (row, head) from a head-on-free rearranged copy of q/k
(lhsT partition base must be 0 on this stack). Softmax denominator via
ones-matmul into spare PSUM columns (output col-tiling), normalization
folded into one DVE multiply producing a zero-padded layout consumed by
a host-padded Wc. Final scatter into [64, 2002] via GPSIMD
local_scatter of hi/lo bf16 halves, summed in fp32.
"""

import sys

if "/opt/trn_rl_repo" not in sys.path:
    sys.path.insert(0, "/opt/trn_rl_repo")

import numpy as np
import ml_dtypes

B = 512
SEQ = 102
EMBED = 256
HEADS = 16
DK = 16
FF = 1024
LAYERS = 3
N_CORES = 8
RPC = B // N_CORES        # rows per core = 64
GR = 8                    # rows per group
GROUPS = RPC // GR        # 8
GT = GR * SEQ             # tokens per group = 816
NH = 2                    # N-halves per group (408 each)
NSZ = GT // NH            # 408
P1 = 1001
OUT_W = 2 * P1            # 2002

_prog_cache = {}


def _pack_k_major(w, ko):
    # [K, M] -> [128, ko, M] with K = ko*128 split as (ko ki)
    K, M = w.shape
    assert K == ko * 128
    return np.ascontiguousarray(w.reshape(ko, 128, M).transpose(1, 0, 2))


def _host_weights(inp):
    f32 = np.float32
    bf16 = ml_dtypes.bfloat16

    def g(name):
        return np.asarray(inp[name], dtype=f32)

    wq, wk, wv = g("Wq"), g("Wk"), g("Wv")      # [3, 256, 256]
    wc, bc = g("Wc"), g("bc")                   # [3, 256, 256], [3, 256]
    w1, b1 = g("W1"), g("b1")                   # [3, 256, 1024], [3, 1024]
    w2, b2 = g("W2"), g("b2")                   # [3, 1024, 256], [3, 256]

    def stack_layers(ws, ko, dt):
        return np.stack([_pack_k_major(w, ko) for w in ws], axis=1).astype(dt)
        # -> [128, 3, ko, M]

    out = {}
    out["wq"] = stack_layers(wq, 2, bf16)       # [128, 3, 2, 256]
    out["wk"] = stack_layers(wk, 2, bf16)
    out["wv"] = stack_layers(wv, 2, bf16)
    # Wc padded: row (c*128 + 32*j + i) = Wc[(4c+j)*16 + i] for i<16 else 0
    wc_pad = np.zeros((3, 512, 256), f32)
    for l in range(3):
        w4 = wc[l].reshape(16, 16, 256)         # [h, i, out]
        for c in range(4):
            for j in range(4):
                wc_pad[l, c * 128 + 32 * j : c * 128 + 32 * j + 16] = w4[4 * c + j]
    out["wc"] = stack_layers(wc_pad, 4, bf16)   # [128, 3, 4, 256]
    out["w1"] = stack_layers(w1, 2, bf16)       # [128, 3, 2, 1024]
    out["w2"] = stack_layers(w2, 8, bf16)       # [128, 3, 8, 256]
    out["bc"] = np.ascontiguousarray(
        bc.reshape(3, 2, 128).transpose(2, 0, 1)).astype(f32)   # [128, 3, 2]
    out["b1"] = np.ascontiguousarray(
        b1.reshape(3, 8, 128).transpose(2, 0, 1)).astype(f32)   # [128, 3, 8]
    out["b2"] = np.ascontiguousarray(
        b2.reshape(3, 2, 128).transpose(2, 0, 1)).astype(f32)   # [128, 3, 2]
    out["wnv"] = _pack_k_major(g("W_nv"), 2).astype(f32)        # [128, 2, 256]
    out["wv2"] = _pack_k_major(g("W_v"), 2).astype(f32)
    out["bnv"] = np.ascontiguousarray(
        g("b_nv").reshape(2, 128).T).astype(f32)                # [128, 2]
    out["bv2"] = np.ascontiguousarray(g("b_v").reshape(2, 128).T).astype(f32)
    out["wf"] = np.ascontiguousarray(
        g("Wf").reshape(2, 128).T).astype(f32)                  # [128, 2]
    return out


def _build_program():
    import concourse.bass as bass
    import concourse.tile as tile
    from concourse import bacc, mybir

    f32 = mybir.dt.float32
    bf16 = mybir.dt.bfloat16

    nc = bacc.Bacc("TRN2", target_bir_lowering=False, debug=False,
                   num_devices=N_CORES)

    def din(name, shape, dt=f32):
        return nc.declare_dram_parameter(name, list(shape), dt, isOutput=False)

    x_d = din("x", [RPC, SEQ, EMBED])
    mask_d = din("mask", [RPC, SEQ])
    idx_d = din("idx", [RPC, 100], mybir.dt.int16)
    wq_d = din("wq", [128, 3, 2, 256], bf16)
    wk_d = din("wk", [128, 3, 2, 256], bf16)
    wv_d = din("wv", [128, 3, 2, 256], bf16)
    wc_d = din("wc", [128, 3, 4, 256], bf16)
    w1_d = din("w1", [128, 3, 2, 1024], bf16)
    w2_d = din("w2", [128, 3, 8, 256], bf16)
    bc_d = din("bc", [128, 3, 2])
    b1_d = din("b1", [128, 3, 8])
    b2_d = din("b2", [128, 3, 2])
    wnv_d = din("wnv", [128, 2, 256])
    wv2_d = din("wv2", [128, 2, 256])
    bnv_d = din("bnv", [128, 2])
    bv2_d = din("bv2", [128, 2])
    wf_d = din("wf", [128, 2])
    out_d = nc.declare_dram_parameter("out", [RPC, OUT_W], f32, isOutput=True)

    lg_dram = nc.dram_tensor("lg_bounce", [RPC, SEQ], f32)

    with tile.TileContext(nc) as tc:
        wpool = tc.alloc_tile_pool(name="w", bufs=1)
        xpool = tc.alloc_tile_pool(name="x", bufs=3)
        apool = tc.alloc_tile_pool(name="a", bufs=2)
        vpool = tc.alloc_tile_pool(name="v", bufs=10)
        hdpool = tc.alloc_tile_pool(name="hd", bufs=1)
        espool = tc.alloc_tile_pool(name="es", bufs=2)
        lin_ps = tc.alloc_tile_pool(name="lps", bufs=2, space="PSUM")
        s_ps = tc.alloc_tile_pool(name="sps", bufs=1, space="PSUM")
        a_ps = tc.alloc_tile_pool(name="aps", bufs=2, space="PSUM")

        # ---- persistent weights ----
        def wtile(dram, shape, dt, tag):
            t = wpool.tile(list(shape), dt, tag=tag)
            nc.sync.dma_start(out=t[:], in_=dram[:])
            return t

        wq = wtile(wq_d, [128, 3, 2, 256], bf16, "wq")
        wk = wtile(wk_d, [128, 3, 2, 256], bf16, "wk")
        wv = wtile(wv_d, [128, 3, 2, 256], bf16, "wv")
        wc = wtile(wc_d, [128, 3, 4, 256], bf16, "wc")
        w1 = wtile(w1_d, [128, 3, 2, 1024], bf16, "w1")
        w2 = wtile(w2_d, [128, 3, 8, 256], bf16, "w2")
        bc = wtile(bc_d, [128, 3, 2], f32, "bc")
        b1 = wtile(b1_d, [128, 3, 8], f32, "b1")
        b2 = wtile(b2_d, [128, 3, 2], f32, "b2")
        wnv = wtile(wnv_d, [128, 2, 256], f32, "wnv")
        wv2 = wtile(wv2_d, [128, 2, 256], f32, "wv2")
        bnv = wtile(bnv_d, [128, 2], f32, "bnv")
        bv2 = wtile(bv2_d, [128, 2], f32, "bv2")
        wf = wtile(wf_d, [128, 2], f32, "wf")
        mask_sb = wtile(mask_d, [RPC, SEQ], f32, "mask")
        idx_sb = wtile(idx_d, [RPC, 100], mybir.dt.int16, "idx")

        ones32 = wpool.tile([128, 32], bf16)
        nc.vector.memset(ones32[:], 1.0)

        # attention psum: fixed tiles, pad rows zeroed once and never
        # rewritten (matmuls only touch [32j, 32j+16) rows)
        attnps = [a_ps.tile([128, 4, SEQ], f32, name=f"attnps{i}", tag="attnps") for i in range(2)]
        for t in attnps:
            nc.vector.memset(t[:], 0.0)

        for g in range(GROUPS):
            b0 = g * GR
            # ---- load x group into feature-major reordered layout ----
            xt = xpool.tile([128, 2, GR, SEQ], f32, tag="xt")
            xg = x_d[b0 : b0 + GR]
            src = xg.rearrange("b n (ko ki) -> ki ko b n", ki=128)
            for ko in range(2):
                for b in range(GR):
                    nc.sync.dma_start(out=xt[:, ko, b, 1:51],
                                      in_=src[:, ko, b, 0:50])
                    nc.sync.dma_start(out=xt[:, ko, b, 52:102],
                                      in_=src[:, ko, b, 51:101])
            lnv = apool.tile([128, 2, GR], f32, tag="lnv")
            lv = apool.tile([128, 2, GR], f32, tag="lv")
            for ko in range(2):
                nc.sync.dma_start(
                    out=lnv[:, ko, :],
                    in_=xg[:, 50, :].rearrange("b (ko ki) -> ki ko b", ki=128)[:, ko, :])
                nc.sync.dma_start(
                    out=lv[:, ko, :],
                    in_=xg[:, 101, :].rearrange("b (ko ki) -> ki ko b", ki=128)[:, ko, :])
            # project tokens 50 -> pos 0 (W_nv), 101 -> pos 51 (W_v)
            for src_t, w_t, b_t, pos in ((lnv, wnv, bnv, 0), (lv, wv2, bv2, 51)):
                ps = lin_ps.tile([128, 2, GR], f32, tag="lin")
                for mo in range(2):
                    for ko in range(2):
                        nc.tensor.matmul(
                            out=ps[:, mo, :],
                            lhsT=w_t[:, ko, mo * 128 : (mo + 1) * 128],
                            rhs=src_t[:, ko, :],
                            start=(ko == 0), stop=(ko == 1))
                for mo in range(2):
                    nc.scalar.activation(
                        out=xt[:, mo, :, pos],
                        in_=ps[:, mo, :],
                        func=mybir.ActivationFunctionType.Identity,
                        bias=b_t[:, mo : mo + 1], scale=1.0)

            # ---- layers ----
            for l in range(LAYERS):
                xbf = apool.tile([128, 2, GR, SEQ], bf16, tag="xbf")
                for ko in range(2):
                    nc.vector.tensor_copy(out=xbf[:, ko], in_=xt[:, ko])

                # Q, K projections (feature-major bf16)
                qbf = apool.tile([128, 2, GR, SEQ], bf16, tag="qbf")
                kbf = apool.tile([128, 2, GR, SEQ], bf16, tag="kbf")
                for w_t, o_t in ((wq, qbf), (wk, kbf)):
                    for mo in range(2):
                        for nh in range(NH):
                            ps = lin_ps.tile([128, NSZ], f32, tag="lin")
                            rr = slice(nh * 4, nh * 4 + 4)
                            for ko in range(2):
                                nc.tensor.matmul(
                                    out=ps[:],
                                    lhsT=w_t[:, l, ko, mo * 128 : (mo + 1) * 128],
                                    rhs=xbf[:, ko, rr],
                                    start=(ko == 0), stop=(ko == 1))
                            nc.scalar.copy(out=o_t[:, mo, rr], in_=ps[:])

                # V token-major per row + bf16
                vbfs = []
                for b in range(GR):
                    ps = lin_ps.tile([SEQ, 256], f32, tag="lin")
                    for ko in range(2):
                        nc.tensor.matmul(
                            out=ps[:],
                            lhsT=xbf[:, ko, b],
                            rhs=wv[:, l, ko, :],
                            start=(ko == 0), stop=(ko == 1))
                    vb = vpool.tile([SEQ, 256], bf16, tag="vbf")
                    nc.scalar.copy(out=vb[:], in_=ps[:])
                    vbfs.append(vb)

                # rearrange q/k to head-on-free [16, HEADS, 4*SEQ] per half
                onrm = apool.tile([128, 4, GR, SEQ], bf16, tag="onrm")
                for half in range(2):
                    qhd = hdpool.tile([16, HEADS, 4 * SEQ], bf16, tag="qhd")
                    khd = hdpool.tile([16, HEADS, 4 * SEQ], bf16, tag="khd")
                    hrr = slice(half * 4, half * 4 + 4)
                    for h in range(HEADS):
                        nc.sync.dma_start(
                            out=qhd[:, h],
                            in_=qbf[16 * (h % 8) : 16 * (h % 8) + 16, h // 8, hrr]
                            .rearrange("d b n -> d (b n)"))
                        nc.sync.dma_start(
                            out=khd[:, h],
                            in_=kbf[16 * (h % 8) : 16 * (h % 8) + 16, h // 8, hrr]
                            .rearrange("d b n -> d (b n)"))
                    for bi in range(4):
                        b = half * 4 + bi
                        brr = slice(bi * SEQ, (bi + 1) * SEQ)
                        sps = s_ps.tile([128, 4, 512], f32, tag="sps")
                        for h in range(HEADS):
                            c, j = h // 4, h % 4
                            nc.tensor.matmul(
                                out=sps[0:SEQ, c, j * SEQ : (j + 1) * SEQ],
                                lhsT=khd[:, h, brr],
                                rhs=qhd[:, h, brr],
                                start=True, stop=True)
                        exps = espool.tile([SEQ, HEADS, SEQ], bf16, tag="exps")
                        nc.scalar.activation(
                            out=exps[:], in_=sps[0:SEQ, :, 0 : 4 * SEQ],
                            func=mybir.ActivationFunctionType.Exp,
                            bias=0.0, scale=0.25)
                        # column sums -> sps[32j:32j+32, :, 408:510]
                        ex_j = exps[:].rearrange("p (c j) n -> p j c n", j=4)
                        for j in range(4):
                            nc.tensor.matmul(
                                out=sps[32 * j : 32 * j + 32, :, 4 * SEQ : 5 * SEQ],
                                lhsT=ones32[0:SEQ, :],
                                rhs=ex_j[:, j],
                                start=True, stop=True,
                                tile_position=(0, 32 * j) if j == 3 else None)
                        rcp = apool.tile([128, 4, SEQ], f32, tag="rcp")
                        nc.vector.reciprocal(
                            out=rcp[:], in_=sps[:, :, 4 * SEQ : 5 * SEQ])
                        aps = attnps[b % 2]
                        for h in range(HEADS):
                            c, j = h // 4, h % 4
                            nc.tensor.matmul(
                                out=aps[32 * j : 32 * j + 16, c, :],
                                lhsT=vbfs[b][:, h * 16 : (h + 1) * 16],
                                rhs=exps[:, h, :],
                                start=True, stop=True,
                                tile_position=(0, 32 * j) if j == 3 else None)
                        nc.vector.tensor_mul(
                            out=onrm[:, :, b, :], in0=aps[:], in1=rcp[:])

                # Wc + bias + residual -> out1 ; bf16 copy
                out1 = apool.tile([128, 2, GR, SEQ], f32, tag="out1")
                o1bf = apool.tile([128, 2, GR, SEQ], bf16, tag="o1bf")
                for mo in range(2):
                    for nh in range(NH):
                        rr = slice(nh * 4, nh * 4 + 4)
                        ps = lin_ps.tile([128, NSZ], f32, tag="lin")
                        for c in range(4):
                            nc.tensor.matmul(
                                out=ps[:],
                                lhsT=wc[:, l, c, mo * 128 : (mo + 1) * 128],
                                rhs=onrm[:, c, rr],
                                start=(c == 0), stop=(c == 3))
                        nc.vector.scalar_tensor_tensor(
                            out=out1[:, mo, rr], in0=ps[:],
                            scalar=bc[:, l, mo : mo + 1],
                            in1=xt[:, mo, rr],
                            op0=mybir.AluOpType.add, op1=mybir.AluOpType.add)
                        nc.vector.tensor_copy(out=o1bf[:, mo, rr],
                                              in_=out1[:, mo, rr])

                # FF: relu(out1 @ W1 + b1) @ W2 + b2 + out1 -> xt (in place)
                hbf = apool.tile([128, 8, GR, SEQ], bf16, tag="hbf")
                for mo in range(8):
                    for nh in range(NH):
                        rr = slice(nh * 4, nh * 4 + 4)
                        ps = lin_ps.tile([128, NSZ], f32, tag="lin")
                        for ko in range(2):
                            nc.tensor.matmul(
                                out=ps[:],
                                lhsT=w1[:, l, ko, mo * 128 : (mo + 1) * 128],
                                rhs=o1bf[:, ko, rr],
                                start=(ko == 0), stop=(ko == 1))
                        nc.scalar.activation(
                            out=hbf[:, mo, rr], in_=ps[:],
                            func=mybir.ActivationFunctionType.Relu,
                            bias=b1[:, l, mo : mo + 1], scale=1.0)
                for mo in range(2):
                    for nh in range(NH):
                        rr = slice(nh * 4, nh * 4 + 4)
                        ps = lin_ps.tile([128, NSZ], f32, tag="lin")
                        for ko in range(8):
                            nc.tensor.matmul(
                                out=ps[:],
                                lhsT=w2[:, l, ko, mo * 128 : (mo + 1) * 128],
                                rhs=hbf[:, ko, rr],
                                start=(ko == 0), stop=(ko == 7))
                        nc.vector.scalar_tensor_tensor(
                            out=xt[:, mo, rr], in0=ps[:],
                            scalar=b2[:, l, mo : mo + 1],
                            in1=out1[:, mo, rr],
                            op0=mybir.AluOpType.add, op1=mybir.AluOpType.add)

            # ---- logits for this group -> DRAM bounce ----
            lgfm = apool.tile([1, GT], f32, tag="lgfm")
            for nh in range(NH):
                rr = slice(nh * 4, nh * 4 + 4)
                ps = lin_ps.tile([1, NSZ], f32, tag="lin")
                for ko in range(2):
                    nc.tensor.matmul(
                        out=ps[:],
                        lhsT=wf[:, ko : ko + 1],
                        rhs=xt[:, ko, rr],
                        start=(ko == 0), stop=(ko == 1))
                nc.scalar.copy(out=lgfm[:, nh * NSZ : (nh + 1) * NSZ], in_=ps[:])
            nc.sync.dma_start(out=lg_dram[b0 : b0 + GR], in_=lgfm[:])

        # ---- epilogue: softmax + where + scatter ----
        lg = wpool.tile([RPC, SEQ], f32)
        nc.sync.dma_start(out=lg[:], in_=lg_dram[:])
        nc.vector.tensor_add(out=lg[:], in0=lg[:], in1=mask_sb[:])
        mx = wpool.tile([RPC, 1], f32)
        nc.vector.tensor_reduce(out=mx[:], in_=lg[:], axis=mybir.AxisListType.X,
                                op=mybir.AluOpType.max, negate=True)
        pexp = wpool.tile([RPC, SEQ], f32)
        ssum = wpool.tile([RPC, 1], f32)
        nc.scalar.activation(out=pexp[:], in_=lg[:],
                             func=mybir.ActivationFunctionType.Exp,
                             bias=mx[:], scale=1.0, accum_out=ssum[:])
        rs = wpool.tile([RPC, 1], f32)
        nc.vector.reciprocal(out=rs[:], in_=ssum[:])
        props = wpool.tile([RPC, SEQ], f32)
        nc.vector.tensor_scalar_mul(out=props[:], in0=pexp[:], scalar1=rs[:])
        small = wpool.tile([RPC, SEQ], f32)
        nc.vector.tensor_scalar(out=small[:], in0=props[:], scalar1=1e-5,
                                scalar2=None, op0=mybir.AluOpType.is_le)
        pc = wpool.tile([RPC, 100], f32)
        for dst, src in ((slice(0, 50), slice(1, 51)), (slice(50, 100), slice(52, 102))):
            nc.vector.scalar_tensor_tensor(
                out=pc[:, dst], in0=small[:, src], scalar=1e-7,
                in1=props[:, src],
                op0=mybir.AluOpType.mult, op1=mybir.AluOpType.add)
        hi = wpool.tile([RPC, 100], bf16)
        nc.vector.tensor_copy(out=hi[:], in_=pc[:])
        hif = wpool.tile([RPC, 100], f32)
        nc.vector.tensor_copy(out=hif[:], in_=hi[:])
        lof = wpool.tile([RPC, 100], f32)
        nc.vector.tensor_tensor(out=lof[:], in0=pc[:], in1=hif[:],
                                op=mybir.AluOpType.subtract)
        lo = wpool.tile([RPC, 100], bf16)
        nc.vector.tensor_copy(out=lo[:], in_=lof[:])
        sc_hi = wpool.tile([RPC, OUT_W], bf16)
        sc_lo = wpool.tile([RPC, OUT_W], bf16)
        nc.gpsimd.local_scatter(out_ap=sc_hi[:], data_ap=hi[:], idxs_ap=idx_sb[:],
                                channels=RPC, num_elems=OUT_W, num_idxs=100)
        nc.gpsimd.local_scatter(out_ap=sc_lo[:], data_ap=lo[:], idxs_ap=idx_sb[:],
                                channels=RPC, num_elems=OUT_W, num_idxs=100)
        outf = wpool.tile([RPC, OUT_W], f32)
        nc.vector.tensor_tensor(out=outf[:], in0=sc_hi[:], in1=sc_lo[:],
                                op=mybir.AluOpType.add)
        nc.vector.tensor_scalar_max(out=outf[:], in0=outf[:], scalar1=1e-20)
        nc.sync.dma_start(out=out_d[:], in_=outf[:])

        a_ps.release()
        s_ps.release()
        lin_ps.release()
        espool.release()
        hdpool.release()
        vpool.release()
        apool.release()
        xpool.release()
        wpool.release()

    nc.compile()
    return nc


def get_program():
    if "nc" not in _prog_cache:
        _prog_cache["nc"] = _build_program()
    return _prog_cache["nc"]


def kernel(**inputs):
    from concourse.bass_utils import run_bass_kernel_spmd

    nc = get_program()
    w = _host_weights(inputs)

    x = np.asarray(inputs["embedded_norm_last_knn_node"], np.float32)
    knn_mask = np.asarray(inputs["knn_node_ninf_mask"], np.float32)
    last = np.asarray(inputs["last_unselect_list"], np.int64)
    depot = np.asarray(inputs["depot_unselect_list"], np.int64)

    mask = np.zeros((B, SEQ), np.float32)
    mask[:, 0] = -1e30
    mask[:, 51] = -1e30
    mask[:, 1:51] = knn_mask
    idx = np.concatenate([last, depot + P1], axis=1).astype(np.int16)

    in_maps = []
    for c in range(N_CORES):
        s = slice(c * RPC, (c + 1) * RPC)
        m = {"x": np.ascontiguousarray(x[s]),
             "mask": np.ascontiguousarray(mask[s]),
             "idx": np.ascontiguousarray(idx[s])}
        m.update(w)
        in_maps.append(m)

    res = run_bass_kernel_spmd(nc, in_maps, core_ids=list(range(N_CORES)))
    return np.concatenate([res.results[c]["out"] for c in range(N_CORES)], axis=0)


# revision 9
# speedup vs baseline: 1.4031x; 1.4031x over previous
"""CVRP decoder (3-layer transformer + scatter) on 8 trn2 NeuronCores.

Self-contained: hardcodes shapes/sharding for
  nn_CVRP_Decoder (B=512, SEQ=102, EMBED=256, HEADS=16, DK=16, FF=1024, L=3).

Strategy: pure data parallel over batch (64 rows/core). Per core the
whole network runs feature-major ([256->2x128 partitions, tokens]) in
groups of 8 batch rows (816 tokens); attention is row-local. Matmuls in
bf16 (fp32 PSUM accumulate), residual stream in fp32. Scores S^T =
k_h^T q_h per (row, head) from a head-on-free rearranged copy of q/k
(lhsT partition base must be 0 on this stack). Softmax denominator via
ones-matmul into spare PSUM columns (output col-tiling), normalization
folded into one DVE multiply producing a zero-padded layout consumed by
a host-padded Wc. Final scatter into [64, 2002] via GPSIMD
local_scatter of hi/lo bf16 halves, summed in fp32.
"""

import sys

if "/opt/trn_rl_repo" not in sys.path:
    sys.path.insert(0, "/opt/trn_rl_repo")

import numpy as np
import ml_dtypes

B = 512
SEQ = 102
EMBED = 256
HEADS = 16
DK = 16
FF = 1024
LAYERS = 3
N_CORES = 8
RPC = B // N_CORES        # rows per core = 64
GR = 8                    # rows per group
GROUPS = RPC // GR        # 8
GT = GR * SEQ             # tokens per group = 816
NH = 2                    # N-halves per group (408 each)
NSZ = GT // NH            # 408
P1 = 1001
OUT_W = 2 * P1            # 2002

_prog_cache = {}


def _pack_k_major(w, ko):
    # [K, M] -> [128, ko, M] with K = ko*128 split as (ko ki)
    K, M = w.shape
    assert K == ko * 128
    return np.ascontiguousarray(w.reshape(ko, 128, M).transpose(1, 0, 2))


def _host_weights(inp):
    f32 = np.float32
    bf16 = ml_dtypes.bfloat16

    def g(name):
        return np.asarray(inp[name], dtype=f32)

    wq, wk, wv = g("Wq"), g("Wk"), g("Wv")      # [3, 256, 256]
    wc, bc = g("Wc"), g("bc")                   # [3, 256, 256], [3, 256]
    w1, b1 = g("W1"), g("b1")                   # [3, 256, 1024], [3, 1024]
    w2, b2 = g("W2"), g("b2")                   # [3, 1024, 256], [3, 256]

    def stack_layers(ws, ko, dt):
        return np.stack([_pack_k_major(w, ko) for w in ws], axis=1).astype(dt)
        # -> [128, 3, ko, M]

    out = {}
    out["wq"] = stack_layers(wq, 2, bf16)       # [128, 3, 2, 256]
    out["wk"] = stack_layers(wk, 2, bf16)
    out["wv"] = stack_layers(wv, 2, bf16)
    # Wc padded: row (c*128 + 32*j + i) = Wc[(4c+j)*16 + i] for i<16 else 0
    wc_pad = np.zeros((3, 512, 256), f32)
    for l in range(3):
        w4 = wc[l].reshape(16, 16, 256)         # [h, i, out]
        for c in range(4):
            for j in range(4):
                wc_pad[l, c * 128 + 32 * j : c * 128 + 32 * j + 16] = w4[4 * c + j]
    out["wc"] = stack_layers(wc_pad, 4, bf16)   # [128, 3, 4, 256]
    out["w1"] = stack_layers(w1, 2, bf16)       # [128, 3, 2, 1024]
    out["w2"] = stack_layers(w2, 8, bf16)       # [128, 3, 8, 256]
    out["bc"] = np.ascontiguousarray(
        bc.reshape(3, 2, 128).transpose(2, 0, 1)).astype(f32)   # [128, 3, 2]
    out["b1"] = np.ascontiguousarray(
        b1.reshape(3, 8, 128).transpose(2, 0, 1)).astype(f32)   # [128, 3, 8]
    out["b2"] = np.ascontiguousarray(
        b2.reshape(3, 2, 128).transpose(2, 0, 1)).astype(f32)   # [128, 3, 2]
    out["wnv"] = _pack_k_major(g("W_nv"), 2).astype(f32)        # [128, 2, 256]
    out["wv2"] = _pack_k_major(g("W_v"), 2).astype(f32)
    out["bnv"] = np.ascontiguousarray(
        g("b_nv").reshape(2, 128).T).astype(f32)                # [128, 2]
    out["bv2"] = np.ascontiguousarray(g("b_v").reshape(2, 128).T).astype(f32)
    out["wf"] = np.ascontiguousarray(
        g("Wf").reshape(2, 128).T).astype(f32)                  # [128, 2]
    return out


def _build_program():
    import concourse.bass as bass
    import concourse.tile as tile
    from concourse import bacc, mybir

    f32 = mybir.dt.float32
    bf16 = mybir.dt.bfloat16

    nc = bacc.Bacc("TRN2", target_bir_lowering=False, debug=False,
                   num_devices=N_CORES)

    def din(name, shape, dt=f32):
        return nc.declare_dram_parameter(name, list(shape), dt, isOutput=False)

    x_d = din("x", [128, 2, RPC, SEQ])
    mask_d = din("mask", [RPC, SEQ])
    idx_d = din("idx", [RPC, 100], mybir.dt.int16)
    wq_d = din("wq", [128, 3, 2, 256], bf16)
    wk_d = din("wk", [128, 3, 2, 256], bf16)
    wv_d = din("wv", [128, 3, 2, 256], bf16)
    wc_d = din("wc", [128, 3, 4, 256], bf16)
    w1_d = din("w1", [128, 3, 2, 1024], bf16)
    w2_d = din("w2", [128, 3, 8, 256], bf16)
    bc_d = din("bc", [128, 3, 2])
    b1_d = din("b1", [128, 3, 8])
    b2_d = din("b2", [128, 3, 2])
    wnv_d = din("wnv", [128, 2, 256])
    wv2_d = din("wv2", [128, 2, 256])
    bnv_d = din("bnv", [128, 2])
    bv2_d = din("bv2", [128, 2])
    wf_d = din("wf", [128, 2])
    out_d = nc.declare_dram_parameter("out", [RPC, OUT_W], f32, isOutput=True)

    lg_dram = nc.dram_tensor("lg_bounce", [RPC, SEQ], f32)

    with tile.TileContext(nc) as tc:
        wpool = tc.alloc_tile_pool(name="w", bufs=1)
        xpool = tc.alloc_tile_pool(name="x", bufs=3)
        apool = tc.alloc_tile_pool(name="a", bufs=2)
        vpool = tc.alloc_tile_pool(name="v", bufs=10)
        hdpool = tc.alloc_tile_pool(name="hd", bufs=1)
        espool = tc.alloc_tile_pool(name="es", bufs=2)
        lin_ps = tc.alloc_tile_pool(name="lps", bufs=2, space="PSUM")
        s_ps = tc.alloc_tile_pool(name="sps", bufs=1, space="PSUM")
        a_ps = tc.alloc_tile_pool(name="aps", bufs=2, space="PSUM")

        # ---- persistent weights ----
        def wtile(dram, shape, dt, tag):
            t = wpool.tile(list(shape), dt, tag=tag)
            nc.sync.dma_start(out=t[:], in_=dram[:])
            return t

        wq = wtile(wq_d, [128, 3, 2, 256], bf16, "wq")
        wk = wtile(wk_d, [128, 3, 2, 256], bf16, "wk")
        wv = wtile(wv_d, [128, 3, 2, 256], bf16, "wv")
        wc = wtile(wc_d, [128, 3, 4, 256], bf16, "wc")
        w1 = wtile(w1_d, [128, 3, 2, 1024], bf16, "w1")
        w2 = wtile(w2_d, [128, 3, 8, 256], bf16, "w2")
        bc = wtile(bc_d, [128, 3, 2], f32, "bc")
        b1 = wtile(b1_d, [128, 3, 8], f32, "b1")
        b2 = wtile(b2_d, [128, 3, 2], f32, "b2")
        wnv = wtile(wnv_d, [128, 2, 256], f32, "wnv")
        wv2 = wtile(wv2_d, [128, 2, 256], f32, "wv2")
        bnv = wtile(bnv_d, [128, 2], f32, "bnv")
        bv2 = wtile(bv2_d, [128, 2], f32, "bv2")
        wf = wtile(wf_d, [128, 2], f32, "wf")
        mask_sb = wtile(mask_d, [RPC, SEQ], f32, "mask")
        idx_sb = wtile(idx_d, [RPC, 100], mybir.dt.int16, "idx")

        ones32 = wpool.tile([128, 32], bf16)
        nc.vector.memset(ones32[:], 1.0)

        # attention psum: fixed tiles, pad rows zeroed once and never
        # rewritten (matmuls only touch [32j, 32j+16) rows)
        attnps = [a_ps.tile([128, 4, SEQ], f32, name=f"attnps{i}", tag="attnps") for i in range(2)]
        for t in attnps:
            nc.vector.memset(t[:], 0.0)

        for g in range(GROUPS):
            b0 = g * GR
            # ---- load x group into feature-major reordered layout ----
            xt = xpool.tile([128, 2, GR, SEQ], f32, tag="xt")
            for ko in range(2):
                nc.sync.dma_start(out=xt[:, ko], in_=x_d[:, ko, b0 : b0 + GR, :])
            # positions 0 / 51 hold raw tokens 50 / 101; project in place
            for w_t, b_t, pos in ((wnv, bnv, 0), (wv2, bv2, 51)):
                ps = lin_ps.tile([128, 2, GR], f32, tag="lin")
                for mo in range(2):
                    for ko in range(2):
                        nc.tensor.matmul(
                            out=ps[:, mo, :],
                            lhsT=w_t[:, ko, mo * 128 : (mo + 1) * 128],
                            rhs=xt[:, ko, :, pos],
                            start=(ko == 0), stop=(ko == 1))
                for mo in range(2):
                    nc.scalar.activation(
                        out=xt[:, mo, :, pos],
                        in_=ps[:, mo, :],
                        func=mybir.ActivationFunctionType.Identity,
                        bias=b_t[:, mo : mo + 1], scale=1.0)

            # ---- layers ----
            for l in range(LAYERS):
                xbf = apool.tile([128, 2, GR, SEQ], bf16, tag="xbf")
                for ko in range(2):
                    nc.vector.tensor_copy(out=xbf[:, ko], in_=xt[:, ko])

                # Q, K projections (feature-major bf16)
                qbf = apool.tile([128, 2, GR, SEQ], bf16, tag="qbf")
                kbf = apool.tile([128, 2, GR, SEQ], bf16, tag="kbf")
                for w_t, o_t in ((wq, qbf), (wk, kbf)):
                    for mo in range(2):
                        for nh in range(NH):
                            ps = lin_ps.tile([128, NSZ], f32, tag="lin")
                            rr = slice(nh * 4, nh * 4 + 4)
                            for ko in range(2):
                                nc.tensor.matmul(
                                    out=ps[:],
                                    lhsT=w_t[:, l, ko, mo * 128 : (mo + 1) * 128],
                                    rhs=xbf[:, ko, rr],
                                    start=(ko == 0), stop=(ko == 1))
                            nc.scalar.copy(out=o_t[:, mo, rr], in_=ps[:])

                # V token-major per row + bf16
                vbfs = []
                for b in range(GR):
                    ps = lin_ps.tile([SEQ, 256], f32, tag="lin")
                    for ko in range(2):
                        nc.tensor.matmul(
                            out=ps[:],
                            lhsT=xbf[:, ko, b],
                            rhs=wv[:, l, ko, :],
                            start=(ko == 0), stop=(ko == 1))
                    vb = vpool.tile([SEQ, 256], bf16, tag="vbf")
                    nc.scalar.copy(out=vb[:], in_=ps[:])
                    vbfs.append(vb)

                # rearrange q/k to head-on-free [16, HEADS, 4*SEQ] per half
                onrm = apool.tile([128, 4, GR, SEQ], bf16, tag="onrm")
                for half in range(2):
                    qhd = hdpool.tile([16, HEADS, 4 * SEQ], bf16, tag="qhd")
                    khd = hdpool.tile([16, HEADS, 4 * SEQ], bf16, tag="khd")
                    hrr = slice(half * 4, half * 4 + 4)
                    for h in range(HEADS):
                        nc.sync.dma_start(
                            out=qhd[:, h],
                            in_=qbf[16 * (h % 8) : 16 * (h % 8) + 16, h // 8, hrr]
                            .rearrange("d b n -> d (b n)"))
                        nc.sync.dma_start(
                            out=khd[:, h],
                            in_=kbf[16 * (h % 8) : 16 * (h % 8) + 16, h // 8, hrr]
                            .rearrange("d b n -> d (b n)"))
                    for bi in range(4):
                        b = half * 4 + bi
                        brr = slice(bi * SEQ, (bi + 1) * SEQ)
                        sps = s_ps.tile([128, 4, 512], f32, tag="sps")
                        for h in range(HEADS):
                            c, j = h // 4, h % 4
                            nc.tensor.matmul(
                                out=sps[0:SEQ, c, j * SEQ : (j + 1) * SEQ],
                                lhsT=khd[:, h, brr],
                                rhs=qhd[:, h, brr],
                                start=True, stop=True)
                        exps = espool.tile([SEQ, HEADS, SEQ], bf16, tag="exps")
                        nc.scalar.activation(
                            out=exps[:], in_=sps[0:SEQ, :, 0 : 4 * SEQ],
                            func=mybir.ActivationFunctionType.Exp,
                            bias=0.0, scale=0.25)
                        # column sums -> sps[32j:32j+32, :, 408:510]
                        ex_j = exps[:].rearrange("p (c j) n -> p j c n", j=4)
                        for j in range(4):
                            nc.tensor.matmul(
                                out=sps[32 * j : 32 * j + 32, :, 4 * SEQ : 5 * SEQ],
                                lhsT=ones32[0:SEQ, :],
                                rhs=ex_j[:, j],
                                start=True, stop=True,
                                tile_position=(0, 32 * j) if j == 3 else None)
                        rcp = apool.tile([128, 4, SEQ], f32, tag="rcp")
                        nc.vector.reciprocal(
                            out=rcp[:], in_=sps[:, :, 4 * SEQ : 5 * SEQ])
                        aps = attnps[b % 2]
                        for h in range(HEADS):
                            c, j = h // 4, h % 4
                            nc.tensor.matmul(
                                out=aps[32 * j : 32 * j + 16, c, :],
                                lhsT=vbfs[b][:, h * 16 : (h + 1) * 16],
                                rhs=exps[:, h, :],
                                start=True, stop=True,
                                tile_position=(0, 32 * j) if j == 3 else None)
                        nc.vector.tensor_mul(
                            out=onrm[:, :, b, :], in0=aps[:], in1=rcp[:])

                # Wc + bias + residual -> out1 ; bf16 copy
                out1 = apool.tile([128, 2, GR, SEQ], f32, tag="out1")
                o1bf = apool.tile([128, 2, GR, SEQ], bf16, tag="o1bf")
                for mo in range(2):
                    for nh in range(NH):
                        rr = slice(nh * 4, nh * 4 + 4)
                        ps = lin_ps.tile([128, NSZ], f32, tag="lin")
                        for c in range(4):
                            nc.tensor.matmul(
                                out=ps[:],
                                lhsT=wc[:, l, c, mo * 128 : (mo + 1) * 128],
                                rhs=onrm[:, c, rr],
                                start=(c == 0), stop=(c == 3))
                        nc.vector.scalar_tensor_tensor(
                            out=out1[:, mo, rr], in0=ps[:],
                            scalar=bc[:, l, mo : mo + 1],
                            in1=xt[:, mo, rr],
                            op0=mybir.AluOpType.add, op1=mybir.AluOpType.add)
                        nc.vector.tensor_copy(out=o1bf[:, mo, rr],
                                              in_=out1[:, mo, rr])

                # FF: relu(out1 @ W1 + b1) @ W2 + b2 + out1 -> xt (in place)
                hbf = apool.tile([128, 8, GR, SEQ], bf16, tag="hbf")
                for mo in range(8):
                    for nh in range(NH):
                        rr = slice(nh * 4, nh * 4 + 4)
                        ps = lin_ps.tile([128, NSZ], f32, tag="lin")
                        for ko in range(2):
                            nc.tensor.matmul(
                                out=ps[:],
                                lhsT=w1[:, l, ko, mo * 128 : (mo + 1) * 128],
                                rhs=o1bf[:, ko, rr],
                                start=(ko == 0), stop=(ko == 1))
                        nc.scalar.activation(
                            out=hbf[:, mo, rr], in_=ps[:],
                            func=mybir.ActivationFunctionType.Relu,
                            bias=b1[:, l, mo : mo + 1], scale=1.0)
                for mo in range(2):
                    for nh in range(NH):
                        rr = slice(nh * 4, nh * 4 + 4)
                        ps = lin_ps.tile([128, NSZ], f32, tag="lin")
                        for ko in range(8):
                            nc.tensor.matmul(
                                out=ps[:],
                                lhsT=w2[:, l, ko, mo * 128 : (mo + 1) * 128],
                                rhs=hbf[:, ko, rr],
                                start=(ko == 0), stop=(ko == 7))
                        nc.vector.scalar_tensor_tensor(
                            out=xt[:, mo, rr], in0=ps[:],
                            scalar=b2[:, l, mo : mo + 1],
                            in1=out1[:, mo, rr],
                            op0=mybir.AluOpType.add, op1=mybir.AluOpType.add)

            # ---- logits for this group -> DRAM bounce ----
            lgfm = apool.tile([1, GT], f32, tag="lgfm")
            for nh in range(NH):
                rr = slice(nh * 4, nh * 4 + 4)
                ps = lin_ps.tile([1, NSZ], f32, tag="lin")
                for ko in range(2):
                    nc.tensor.matmul(
                        out=ps[:],
                        lhsT=wf[:, ko : ko + 1],
                        rhs=xt[:, ko, rr],
                        start=(ko == 0), stop=(ko == 1))
                nc.scalar.copy(out=lgfm[:, nh * NSZ : (nh + 1) * NSZ], in_=ps[:])
            nc.sync.dma_start(out=lg_dram[b0 : b0 + GR], in_=lgfm[:])

        # ---- epilogue: softmax + where + scatter ----
        lg = wpool.tile([RPC, SEQ], f32)
        nc.sync.dma_start(out=lg[:], in_=lg_dram[:])
        nc.vector.tensor_add(out=lg[:], in0=lg[:], in1=mask_sb[:])
        mx = wpool.tile([RPC, 1], f32)
        nc.vector.tensor_reduce(out=mx[:], in_=lg[:], axis=mybir.AxisListType.X,
                                op=mybir.AluOpType.max, negate=True)
        pexp = wpool.tile([RPC, SEQ], f32)
        ssum = wpool.tile([RPC, 1], f32)
        nc.scalar.activation(out=pexp[:], in_=lg[:],
                             func=mybir.ActivationFunctionType.Exp,
                             bias=mx[:], scale=1.0, accum_out=ssum[:])
        rs = wpool.tile([RPC, 1], f32)
        nc.vector.reciprocal(out=rs[:], in_=ssum[:])
        props = wpool.tile([RPC, SEQ], f32)
        nc.vector.tensor_scalar_mul(out=props[:], in0=pexp[:], scalar1=rs[:])
        small = wpool.tile([RPC, SEQ], f32)
        nc.vector.tensor_scalar(out=small[:], in0=props[:], scalar1=1e-5,
                                scalar2=None, op0=mybir.AluOpType.is_le)
        pc = wpool.tile([RPC, 100], f32)
        for dst, src in ((slice(0, 50), slice(1, 51)), (slice(50, 100), slice(52, 102))):
            nc.vector.scalar_tensor_tensor(
                out=pc[:, dst], in0=small[:, src], scalar=1e-7,
                in1=props[:, src],
                op0=mybir.AluOpType.mult, op1=mybir.AluOpType.add)
        hi = wpool.tile([RPC, 100], bf16)
        nc.vector.tensor_copy(out=hi[:], in_=pc[:])
        hif = wpool.tile([RPC, 100], f32)
        nc.vector.tensor_copy(out=hif[:], in_=hi[:])
        lof = wpool.tile([RPC, 100], f32)
        nc.vector.tensor_tensor(out=lof[:], in0=pc[:], in1=hif[:],
                                op=mybir.AluOpType.subtract)
        lo = wpool.tile([RPC, 100], bf16)
        nc.vector.tensor_copy(out=lo[:], in_=lof[:])
        sc_hi = wpool.tile([RPC, OUT_W], bf16)
        sc_lo = wpool.tile([RPC, OUT_W], bf16)
        nc.gpsimd.local_scatter(out_ap=sc_hi[:], data_ap=hi[:], idxs_ap=idx_sb[:],
                                channels=RPC, num_elems=OUT_W, num_idxs=100)
        nc.gpsimd.local_scatter(out_ap=sc_lo[:], data_ap=lo[:], idxs_ap=idx_sb[:],
                                channels=RPC, num_elems=OUT_W, num_idxs=100)
        outf = wpool.tile([RPC, OUT_W], f32)
        nc.vector.tensor_tensor(out=outf[:], in0=sc_hi[:], in1=sc_lo[:],
                                op=mybir.AluOpType.add)
        nc.vector.tensor_scalar_max(out=outf[:], in0=outf[:], scalar1=1e-20)
        nc.sync.dma_start(out=out_d[:], in_=outf[:])

        a_ps.release()
        s_ps.release()
        lin_ps.release()
        espool.release()
        hdpool.release()
        vpool.release()
        apool.release()
        xpool.release()
        wpool.release()

    nc.compile()
    return nc


def get_program():
    if "nc" not in _prog_cache:
        _prog_cache["nc"] = _build_program()
    return _prog_cache["nc"]


def kernel(**inputs):
    from concourse.bass_utils import run_bass_kernel_spmd

    nc = get_program()
    w = _host_weights(inputs)

    x = np.asarray(inputs["embedded_norm_last_knn_node"], np.float32)
    perm = np.concatenate([[50], np.arange(0, 50), [101], np.arange(51, 101)])
    x_re = np.ascontiguousarray(
        x[:, perm, :].transpose(2, 0, 1).reshape(2, 128, B, SEQ).swapaxes(0, 1))
    knn_mask = np.asarray(inputs["knn_node_ninf_mask"], np.float32)
    last = np.asarray(inputs["last_unselect_list"], np.int64)
    depot = np.asarray(inputs["depot_unselect_list"], np.int64)

    mask = np.zeros((B, SEQ), np.float32)
    mask[:, 0] = -1e30
    mask[:, 51] = -1e30
    mask[:, 1:51] = knn_mask
    idx = np.concatenate([last, depot + P1], axis=1).astype(np.int16)

    in_maps = []
    for c in range(N_CORES):
        s = slice(c * RPC, (c + 1) * RPC)
        m = {"x": np.ascontiguousarray(x_re[:, :, s, :]),
             "mask": np.ascontiguousarray(mask[s]),
             "idx": np.ascontiguousarray(idx[s])}
        m.update(w)
        in_maps.append(m)

    res = run_bass_kernel_spmd(nc, in_maps, core_ids=list(range(N_CORES)))
    return np.concatenate([res.results[c]["out"] for c in range(N_CORES)], axis=0)
